# revision 4
# baseline (speedup 1.0000x reference)
"""PhaseEncoding kernel for Trainium2 (8-core SPMD).

Math: out[b,d,s] = x[b,d,s] + sum_f phase_one_hot[b,f,s] * emb_weight[f,d]
Shapes: x (16,512,4096) f32, phase_one_hot (16,9,4096) f32, emb_weight (9,512) f32.
Sharding: batch data-parallel, 2 batches per core; emb_weight replicated.

The kernel is HBM-bandwidth bound (360 GB/s/core aggregate DMA), so all
device I/O is fp16: the host rounds x/poh/w to fp16 (rel rms error ~3e-4,
far inside the output tolerance) and upcasts the fp16 result to f32.
Per-core traffic: 8.4 MB x in + 8.4 MB out + 0.15 MB poh -> ~47 us roofline.

Per [128, 512] tile, two accumulating fp16 matmuls build x + poh@w in
PSUM: the phase contraction (9-deep) plus an identity matmul that streams
the x tile through the PE. A single copy (alternating DVE/Act) evicts
PSUM to the fp16 output tile, keeping every compute engine far below the
DMA roofline.
"""

import numpy as np

B, F, S, D = 16, 9, 4096, 512
NCORES = 8
BPC = B // NCORES  # batches per core

_NC = None


def _build_nc():
    from contextlib import ExitStack

    import concourse.bass as bass
    import concourse.tile as tile
    from concourse import bacc, mybir

    f32 = mybir.dt.float32
    f16 = mybir.dt.float16
    nc = bacc.Bacc(
        "TRN2", target_bir_lowering=False, debug=False, num_devices=NCORES
    )

    x_d = nc.declare_dram_parameter("x", [BPC, D, S], f16, isOutput=False)
    poh_d = nc.declare_dram_parameter("poh", [BPC, F, S], f16, isOutput=False)
    w_d = nc.declare_dram_parameter("emb", [F, D], f16, isOutput=False)
    id_d = nc.declare_dram_parameter("ident", [128, 128], f16, isOutput=False)
    out_d = nc.declare_dram_parameter("out", [BPC, D, S], f16, isOutput=True)

    DC = D // 128  # 4 d-chunks of 128 partitions
    ST = S // 512  # 8 s-tiles of 512 columns
    SH = S // 2  # half-width for DMA splitting

    with tile.TileContext(nc) as tc, ExitStack() as ctx:
        const_pool = ctx.enter_context(tc.tile_pool(name="const", bufs=1))
        poh_pool = ctx.enter_context(tc.tile_pool(name="poh", bufs=1))
        x_pool = ctx.enter_context(tc.tile_pool(name="x", bufs=8))
        o_pool = ctx.enter_context(tc.tile_pool(name="o", bufs=8))
        psum_pool = ctx.enter_context(
            tc.tile_pool(name="psum", bufs=8, space=bass.MemorySpace.PSUM)
        )

        # Small constants go out first on the Act DGE queue so the first
        # matmul's operands land while x half-load 0 is still in flight.
        w_t = const_pool.tile([F, D], f16)
        nc.scalar.dma_start(w_t[:], w_d[:])
        id_t = const_pool.tile([128, 128], f16)
        nc.scalar.dma_start(id_t[:], id_d[:])
        poh_ts = []
        for b in range(BPC):
            p_t = poh_pool.tile([F, S], f16)
            nc.scalar.dma_start(p_t[:], poh_d[b])
            poh_ts.append(p_t)

        # All x loads stream on the SP HWDGE queue; halves so compute can
        # begin mid-tile. SBUF holds all 8 x tiles + 8 out tiles (~128 KB
        # of the 208 KB partition budget), so no load ever waits on a slot.
        x_ts = {}
        for b in range(BPC):
            for dc in range(DC):
                x_t = x_pool.tile([128, S], f16)
                nc.sync.dma_start(x_t[:, :SH], x_d[b, bass.ts(dc, 128), :SH])
                nc.sync.dma_start(x_t[:, SH:], x_d[b, bass.ts(dc, 128), SH:])
                x_ts[(b, dc)] = x_t

        ei = 0
        for b in range(BPC):
            for dc in range(DC):
                x_t = x_ts[(b, dc)]
                o_t = o_pool.tile([128, S], f16)
                for st in range(ST):
                    # Explicit per-bank tags force a strict 8-deep rotation:
                    # the default slot assignment reuses a just-freed bank,
                    # making each matmul wait on the eviction only ~3 tiles
                    # back and serializing the tail of the pipeline.
                    ps = psum_pool.tile(
                        [128, 512], f32, tag=f"ps{ei % 8}", bufs=1, name=f"ps{ei % 8}"
                    )
                    nc.tensor.matmul(
                        ps[:],
                        w_t[:, bass.ts(dc, 128)],
                        poh_ts[b][:, bass.ts(st, 512)],
                        start=True,
                        stop=False,
                    )
                    nc.tensor.matmul(
                        ps[:],
                        id_t[:],
                        x_t[:, bass.ts(st, 512)],
                        start=False,
                        stop=True,
                    )
                    if ei % 2 == 0:
                        nc.vector.tensor_copy(o_t[:, bass.ts(st, 512)], ps[:])
                    else:
                        nc.scalar.activation(
                            o_t[:, bass.ts(st, 512)],
                            ps[:],
                            mybir.ActivationFunctionType.Copy,
                        )
                    ei += 1
                    if st == ST // 2 - 1:
                        nc.scalar.dma_start(
                            out_d[b, bass.ts(dc, 128), :SH], o_t[:, :SH]
                        )
                nc.scalar.dma_start(
                    out_d[b, bass.ts(dc, 128), SH:], o_t[:, SH:]
                )

    nc.compile()
    return nc


def _get_nc():
    global _NC
    if _NC is None:
        _NC = _build_nc()
    return _NC


def kernel(**inputs):
    from concourse.bass_utils import run_bass_kernel_spmd

    x = inputs["x"].astype(np.float16)
    poh = inputs["phase_one_hot"].astype(np.float16)
    w = inputs["emb_weight"].astype(np.float16)
    ident = np.eye(128, dtype=np.float16)

    nc = _get_nc()
    in_maps = [
        {
            "x": np.ascontiguousarray(x[i * BPC : (i + 1) * BPC]),
            "poh": np.ascontiguousarray(poh[i * BPC : (i + 1) * BPC]),
            "emb": w,
            "ident": ident,
        }
        for i in range(NCORES)
    ]
    res = run_bass_kernel_spmd(nc, in_maps, core_ids=list(range(NCORES)))
    out = np.concatenate(
        [np.asarray(res.results[i]["out"]) for i in range(NCORES)], axis=0
    )
    return out.astype(np.float32)


# revision 5
# speedup vs baseline: 1.0727x; 1.0727x over previous
"""PhaseEncoding kernel for Trainium2 (8-core SPMD).

Math: out[b,d,s] = x[b,d,s] + sum_f phase_one_hot[b,f,s] * emb_weight[f,d]
Shapes: x (16,512,4096) f32, phase_one_hot (16,9,4096) f32, emb_weight (9,512) f32.
Sharding: batch data-parallel, 2 batches per core; emb_weight replicated.

The kernel is HBM-bandwidth bound (360 GB/s/core aggregate DMA), so all
device I/O is fp16: the host rounds x/poh/w to fp16 (rel rms error ~3e-4,
far inside the output tolerance) and upcasts the fp16 result to f32.
Per-core traffic: 8.4 MB x in + 8.4 MB out + 0.15 MB poh -> ~47 us roofline.

Per [128, 512] tile, two accumulating fp16 matmuls build x + poh@w in
PSUM: the phase contraction (9-deep) plus an identity matmul that streams
the x tile through the PE. A single copy (alternating DVE/Act) evicts
PSUM to the fp16 output tile, keeping every compute engine far below the
DMA roofline.
"""

import numpy as np

B, F, S, D = 16, 9, 4096, 512
NCORES = 8
BPC = B // NCORES  # batches per core

_NC = None


def _build_nc():
    from contextlib import ExitStack

    import concourse.bass as bass
    import concourse.tile as tile
    from concourse import bacc, mybir

    f32 = mybir.dt.float32
    f16 = mybir.dt.float16
    nc = bacc.Bacc(
        "TRN2", target_bir_lowering=False, debug=False, num_devices=NCORES
    )

    x_d = nc.declare_dram_parameter("x", [BPC, D, S], f16, isOutput=False)
    poh_d = nc.declare_dram_parameter("poh", [BPC, F, S], f16, isOutput=False)
    w_d = nc.declare_dram_parameter("emb", [F, D], f16, isOutput=False)
    id_d = nc.declare_dram_parameter("ident", [128, 128], f16, isOutput=False)
    out_d = nc.declare_dram_parameter("out", [BPC, D, S], f16, isOutput=True)

    DC = D // 128  # 4 d-chunks of 128 partitions
    ST = S // 512  # 8 s-tiles of 512 columns
    SH = S // 2  # half-width for DMA splitting

    with tile.TileContext(nc) as tc, ExitStack() as ctx:
        const_pool = ctx.enter_context(tc.tile_pool(name="const", bufs=1))
        poh_pool = ctx.enter_context(tc.tile_pool(name="poh", bufs=1))
        x_pool = ctx.enter_context(tc.tile_pool(name="x", bufs=8))
        o_pool = ctx.enter_context(tc.tile_pool(name="o", bufs=8))
        psum_pool = ctx.enter_context(
            tc.tile_pool(name="psum", bufs=8, space=bass.MemorySpace.PSUM)
        )

        # Small constants go out first on the Act DGE queue so the first
        # matmul's operands land while x half-load 0 is still in flight.
        w_t = const_pool.tile([F, D], f16)
        nc.scalar.dma_start(w_t[:], w_d[:])
        id_t = const_pool.tile([128, 128], f16)
        nc.scalar.dma_start(id_t[:], id_d[:])
        poh_ts = []
        for b in range(BPC):
            p_t = poh_pool.tile([F, S], f16)
            nc.scalar.dma_start(p_t[:], poh_d[b])
            poh_ts.append(p_t)

        # All x loads stream on the SP HWDGE queue; halves so compute can
        # begin mid-tile. SBUF holds all 8 x tiles + 8 out tiles (~128 KB
        # of the 208 KB partition budget), so no load ever waits on a slot.
        x_ts = {}
        for b in range(BPC):
            for dc in range(DC):
                x_t = x_pool.tile([128, S], f16)
                nc.sync.dma_start(x_t[:, :SH], x_d[b, bass.ts(dc, 128), :SH])
                nc.sync.dma_start(x_t[:, SH:], x_d[b, bass.ts(dc, 128), SH:])
                x_ts[(b, dc)] = x_t

        ei = 0
        for b in range(BPC):
            for dc in range(DC):
                x_t = x_ts[(b, dc)]
                o_t = o_pool.tile([128, S], f16)
                for st in range(ST):
                    # Explicit per-bank tags force a strict 8-deep rotation:
                    # the default slot assignment reuses a just-freed bank,
                    # making each matmul wait on the eviction only ~3 tiles
                    # back and serializing the tail of the pipeline.
                    ps = psum_pool.tile(
                        [128, 512], f32, tag=f"ps{ei % 8}", bufs=1, name=f"ps{ei % 8}"
                    )
                    nc.tensor.matmul(
                        ps[:],
                        w_t[:, bass.ts(dc, 128)],
                        poh_ts[b][:, bass.ts(st, 512)],
                        start=True,
                        stop=False,
                    )
                    nc.tensor.matmul(
                        ps[:],
                        id_t[:],
                        x_t[:, bass.ts(st, 512)],
                        start=False,
                        stop=True,
                    )
                    if ei % 2 == 0:
                        nc.vector.tensor_copy(o_t[:, bass.ts(st, 512)], ps[:])
                    else:
                        nc.scalar.activation(
                            o_t[:, bass.ts(st, 512)],
                            ps[:],
                            mybir.ActivationFunctionType.Copy,
                        )
                    ei += 1
                    # Stores ride the SP queue behind all x loads: the
                    # in-order queue front-loads the x stream (so compute
                    # never starves late) and keeps store dispatch off the
                    # Activation sequencer, which the evictions need.
                    if st == ST // 2 - 1:
                        nc.sync.dma_start(
                            out_d[b, bass.ts(dc, 128), :SH], o_t[:, :SH]
                        )
                nc.sync.dma_start(
                    out_d[b, bass.ts(dc, 128), SH:], o_t[:, SH:]
                )

    nc.compile()
    return nc


def _get_nc():
    global _NC
    if _NC is None:
        _NC = _build_nc()
    return _NC


def kernel(**inputs):
    from concourse.bass_utils import run_bass_kernel_spmd

    x = inputs["x"].astype(np.float16)
    poh = inputs["phase_one_hot"].astype(np.float16)
    w = inputs["emb_weight"].astype(np.float16)
    ident = np.eye(128, dtype=np.float16)

    nc = _get_nc()
    in_maps = [
        {
            "x": np.ascontiguousarray(x[i * BPC : (i + 1) * BPC]),
            "poh": np.ascontiguousarray(poh[i * BPC : (i + 1) * BPC]),
            "emb": w,
            "ident": ident,
        }
        for i in range(NCORES)
    ]
    res = run_bass_kernel_spmd(nc, in_maps, core_ids=list(range(NCORES)))
    out = np.concatenate(
        [np.asarray(res.results[i]["out"]) for i in range(NCORES)], axis=0
    )
    return out.astype(np.float32)
